# revision 1
# baseline (speedup 1.0000x reference)
"""NodeContrastiveLoss Trainium2 kernel.

Full inputs -> scalar loss, data-parallel over 8 NeuronCores (256 batches/core).

Per batch b (reference semantics):
  sums[f,d]  = segment-sum of atom_embed over atom2frag
  mn         = sums / max(||sums||, eps)        (== means/||means|| since count scale cancels)
  fn         = frag / max(||frag||, eps)        (host-precomputed)
  sims       = 10 * mn @ fn.T
  per_frag   = logsumexp(sims, -1) - diag(sims)
  loss       = sum(valid * per_frag) / max(n_valid, 1)

Device pipeline per 8-batch iteration (4 pairs of 2 batches stacked on 128 partitions):
  gpsimd : one-hot = (iota == idx) as bf16
  PE     : 16 seg-matmuls (one-hot.T @ atoms) -> PSUM sums [128,4,128]
  ACT    : copy sums -> SBUF bf16
  DVE    : TTR ssq, clamp; build D = diag(10*rsqrt(ssq)) via ident*scale
  ACT    : ln/exp small ops for the scale (sqrt avoided: stays in one ACT table set)
  PE     : 4 transpose-matmuls sums.T @ D -> scaled mnT (fused transpose+normalize)
  ACT    : copy mnT -> SBUF
  PE     : 4 sims matmuls mnT.T @ fragT_hat -> PSUM [128,4,128]
  ACT    : exp(sims) -> SBUF bf16
  DVE    : TTR x8 (halfmask -> row sums s, identity -> exp(pos)); per_frag = ln(s/spos)
  DVE    : masked per-frag -> accumulation slab; final reduce -> out[128,1]
Host: sum 8x128 partials, divide by n_valid.
"""

import sys

sys.path.insert(0, "/opt/trn_rl_repo")

from contextlib import ExitStack

import ml_dtypes
import numpy as np

import concourse.bacc as bacc
import concourse.bass as bass
import concourse.tile as tile
from concourse import mybir
from concourse.bass_utils import run_bass_kernel_spmd

B, A, F_, D = 2048, 256, 64, 128
NCORES = 8
BPC = B // NCORES          # 256 batches per core
BPI = 8                    # batches per iteration
ITERS = BPC // BPI         # 32
PAIRS = BPI // 2           # 4
CHUNKS = BPI * 2           # 16 chunks of 128 atoms

BF16 = mybir.dt.bfloat16
F32 = mybir.dt.float32
U8 = mybir.dt.uint8
I32 = mybir.dt.int32
ALU = mybir.AluOpType
ACTF = mybir.ActivationFunctionType
AXIS = mybir.AxisListType

LN10 = float(np.log(10.0))


def build_body(tc, outs, ins):
    """Tile kernel body. ins/outs: dicts of DRAM APs.

    ins: atoms [ITERS,128,CHUNKS*128] bf16, fragT [ITERS,128,PAIRS*128] bf16,
         idx [ITERS,128,CHUNKS] u8, mask [ITERS,128,PAIRS] bf16
    outs: out [128,1] f32
    """
    nc = tc.nc
    ctx = ExitStack()
    with ctx:
        const = ctx.enter_context(tc.tile_pool(name="const", bufs=1))
        dpool = ctx.enter_context(tc.tile_pool(name="dma", bufs=4))
        work = ctx.enter_context(tc.tile_pool(name="work", bufs=3))
        small = ctx.enter_context(tc.tile_pool(name="small", bufs=3))
        pseg = ctx.enter_context(tc.tile_pool(name="pseg", bufs=2, space="PSUM"))
        ptr = ctx.enter_context(tc.tile_pool(name="ptr", bufs=2, space="PSUM"))
        psim = ctx.enter_context(tc.tile_pool(name="psim", bufs=2, space="PSUM"))

        # ---- one-time constants ----
        iota_i32 = const.tile([128, CHUNKS, 64], I32)
        nc.gpsimd.iota(iota_i32[:], [[0, CHUNKS], [1, 64]], channel_multiplier=0)
        iota_bf = const.tile([128, CHUNKS, 64], BF16)
        nc.gpsimd.tensor_copy(iota_bf[:], iota_i32[:])

        # identity (bf16): ident[p,f] = (p == f)
        irow = const.tile([128, 128], I32)
        nc.gpsimd.iota(irow[:], [[1, 128]], channel_multiplier=0)
        icol = const.tile([128, 1], I32)
        nc.gpsimd.iota(icol[:], [[0, 1]], channel_multiplier=1)
        irow_f = const.tile([128, 128], F32)
        nc.gpsimd.tensor_copy(irow_f[:], irow[:])
        icol_f = const.tile([128, 1], F32)
        nc.gpsimd.tensor_copy(icol_f[:], icol[:])
        ident = const.tile([128, 128], BF16)
        nc.vector.tensor_scalar(ident[:], irow_f[:], icol_f[:], None, op0=ALU.is_equal)

        # accumulation slab: [128, ITERS, PAIRS] f32, fully written each run
        slab = const.tile([128, ITERS, PAIRS], F32)

        ln10_t = const.tile([128, 1], F32)
        nc.gpsimd.memset(ln10_t[:], LN10)

        # all iters' idx+mask metadata in one upfront DMA
        meta_all = const.tile([128, ITERS, 24], U8)
        nc.sync.dma_start(meta_all[:], ins["meta"])

        for it in range(ITERS):
            # ---- DMA in: big contiguous slabs, dispatch spread across
            # engines (SP's DGE-config cost ~600ns/dma was the v1 gate) ----
            atoms_t = dpool.tile([128, CHUNKS, 128], BF16, tag="atoms")
            nc.sync.dma_start(
                atoms_t[:].rearrange("p c d -> p (c d)"), ins["atoms"][it]
            )
            fragT_t = dpool.tile([128, PAIRS, 128], BF16, tag="fragT")
            nc.sync.dma_start(
                fragT_t[:].rearrange("p j d -> p (j d)"), ins["fragT"][it]
            )
            idx_t = meta_all[:, it, 0:16]
            mask_t = meta_all[:, it, 16:24].bitcast(BF16)

            # ---- one-hot ----
            idx_bf = small.tile([128, CHUNKS], BF16, tag="idxbf")
            nc.gpsimd.tensor_copy(idx_bf[:], idx_t)
            onehot = work.tile([128, CHUNKS, 64], BF16, tag="onehot")
            idx_bc = idx_bf[:].unsqueeze(2).broadcast_to((128, CHUNKS, 64))
            nc.vector.tensor_tensor(onehot[:], iota_bf[:], idx_bc, op=ALU.is_equal)

            # ---- segment-sum matmuls: psum_seg[64*tw:+64, j, :] += onehot_c.T @ atoms_c
            psum_seg = pseg.tile([128, PAIRS, 128], F32, tag="seg")
            for c in range(CHUNKS):
                bi, h = divmod(c, 2)
                j, tw = divmod(bi, 2)
                nc.tensor.matmul(
                    psum_seg[64 * tw : 64 * tw + 64, j, :],
                    onehot[:, c, :],
                    atoms_t[:, c, :],
                    start=(h == 0),
                    stop=(h == 1),
                )

            # ---- sums -> SBUF bf16 (ACT copy) ----
            sums_s = work.tile([128, PAIRS, 128], BF16, tag="sums")
            nc.scalar.copy(sums_s[:], psum_seg[:])

            # ---- ssq + normalization scale: scale = 10 / max(||sums||, 1e-8)
            sq_s = work.tile([128, PAIRS, 128], BF16, tag="sqs")
            nc.vector.tensor_tensor(sq_s[:], sums_s[:], sums_s[:], op=ALU.mult)
            ssq = small.tile([128, PAIRS], F32, tag="ssq")
            nc.vector.tensor_reduce(ssq[:], sq_s[:], axis=AXIS.X, op=ALU.add)
            ssq_c = small.tile([128, PAIRS], F32, tag="ssqc")
            nc.vector.tensor_scalar(ssq_c[:], ssq[:], 1e-16, None, op0=ALU.max)
            lnssq = small.tile([128, PAIRS], F32, tag="lnssq")
            nc.scalar.activation(lnssq[:], ssq_c[:], ACTF.Ln)
            scale_mn = small.tile([128, PAIRS], F32, tag="scale")
            # exp(-0.5*ln(ssq) + ln(10)) = 10 * rsqrt(ssq)
            nc.scalar.activation(
                scale_mn[:], lnssq[:], ACTF.Exp, bias=ln10_t[:], scale=-0.5
            )

            # ---- D = diag(scale): ident row p has its single 1 at col p ----
            D_t = work.tile([128, PAIRS, 128], BF16, tag="D")
            for j in range(PAIRS):
                nc.vector.tensor_scalar(
                    D_t[:, j, :], ident[:], scale_mn[:, j : j + 1], None, op0=ALU.mult
                )

            # ---- fused transpose+normalize (regular matmul): mnT = sums.T @ D
            psum_tr = ptr.tile([128, PAIRS, 128], F32, tag="tr")
            for j in range(PAIRS):
                nc.tensor.matmul(
                    psum_tr[:, j, :],
                    sums_s[:, j, :],
                    D_t[:, j, :],
                    start=True,
                    stop=True,
                )
            mnT_s = work.tile([128, PAIRS, 128], BF16, tag="mnT")
            nc.scalar.copy(mnT_s[:], psum_tr[:])

            # ---- sims matmuls (rows already normalized & x10) ----
            psum_sims = psim.tile([128, PAIRS, 128], F32, tag="sims")
            for j in range(PAIRS):
                nc.tensor.matmul(
                    psum_sims[:, j, :],
                    mnT_s[:, j, :],
                    fragT_t[:, j, :],
                    start=True,
                    stop=True,
                )

            # ---- exp over all sims (one plain ACT op) ----
            exp_s = work.tile([128, PAIRS, 128], BF16, tag="exps")
            nc.scalar.activation(exp_s[:], psum_sims[:], ACTF.Exp)

            # ---- exp(pos) = diag(exp_s) via strided DMA (off compute engines) ----
            epos = small.tile([128, PAIRS], BF16, tag="epos")
            e_h = exp_s[:, 0, 0]
            diag_ap = bass.AP(
                tensor=e_h.tensor,
                offset=e_h.offset,
                ap=[[PAIRS * 128 + 1, 128], [128, PAIRS]],
            )
            nc.gpsimd.dma_start(epos[:], diag_ap)

            # ---- per-half row sums, select own half ----
            s_half = small.tile([128, PAIRS, 2], F32, tag="shalf")
            exp_v = exp_s[:].rearrange("p j (two g) -> p j two g", two=2)
            nc.vector.tensor_reduce(s_half[:], exp_v, axis=AXIS.X, op=ALU.add)
            s_sel = small.tile([128, PAIRS], F32, tag="ssel")
            nc.vector.tensor_copy(s_sel[0:64, :], s_half[0:64, :, 0])
            nc.vector.tensor_copy(s_sel[64:128, :], s_half[64:128, :, 1])

            # ---- per_frag = ln(s_sel / exp(pos)); masked into slab ----
            inv_ep = small.tile([128, PAIRS], F32, tag="invep")
            nc.vector.reciprocal(inv_ep[:], epos[:])
            ratio = small.tile([128, PAIRS], F32, tag="ratio")
            nc.vector.tensor_tensor(ratio[:], s_sel[:], inv_ep[:], op=ALU.mult)
            perfrag = small.tile([128, PAIRS], F32, tag="perfrag")
            nc.scalar.activation(perfrag[:], ratio[:], ACTF.Ln)
            nc.vector.tensor_tensor(
                slab[:, it, :], perfrag[:], mask_t, op=ALU.mult
            )

        # ---- final reduce + output ----
        outsb = const.tile([128, 1], F32)
        nc.vector.tensor_reduce(outsb[:], slab[:], axis=AXIS.XY, op=ALU.add)
        nc.sync.dma_start(outs["out"], outsb[:])


def prep_inputs(atom_embed, fragment_embed, atom2frag):
    """Host-side layout prep. Returns (in_maps, n_valid)."""
    bf = ml_dtypes.bfloat16
    am = np.asarray(atom_embed, dtype=np.float32)
    fe = np.asarray(fragment_embed, dtype=np.float32)
    af = np.asarray(atom2frag)

    # atoms: [B,A,D] -> [core, it, p, (bi,h), d]
    a6 = am.reshape(NCORES, ITERS, BPI, 2, 128, D)
    atoms_np = np.ascontiguousarray(a6.transpose(0, 1, 4, 2, 3, 5)).reshape(
        NCORES, ITERS, 128, CHUNKS * 128
    ).astype(bf)

    # frag normalized + transposed: [core, it, d, (j, tw*64+g)]
    fen = fe / np.maximum(np.linalg.norm(fe, axis=-1, keepdims=True), 1e-8)
    f6 = fen.reshape(NCORES, ITERS, PAIRS, 2, F_, D)
    fragT_np = np.ascontiguousarray(f6.transpose(0, 1, 5, 2, 3, 4)).reshape(
        NCORES, ITERS, 128, PAIRS * 128
    ).astype(bf)

    # idx: [core, it, p, (bi,h)] u8
    i5 = af.reshape(NCORES, ITERS, BPI, 2, 128)
    idx_np = np.ascontiguousarray(i5.transpose(0, 1, 4, 2, 3)).reshape(
        NCORES, ITERS, 128, CHUNKS
    ).astype(np.uint8)

    # counts/valid on host (index metadata)
    counts = (af[:, :, None] == np.arange(F_)[None, None, :]).sum(axis=1)
    valid = counts > 0
    n_valid = int(valid.sum())
    v6 = valid.reshape(NCORES, ITERS, PAIRS, 2, F_)
    mask_np = np.ascontiguousarray(v6.transpose(0, 1, 3, 4, 2)).reshape(
        NCORES, ITERS, 128, PAIRS
    ).astype(bf)

    # pack idx (16B) + mask-as-bytes (8B) into one small tensor,
    # partition-major for a single contiguous upfront DMA
    meta_np = np.concatenate(
        [idx_np, mask_np.view(np.uint8)], axis=-1
    )  # [NCORES, ITERS, 128, 24]
    meta_np = np.ascontiguousarray(meta_np.transpose(0, 2, 1, 3))  # [NC,128,IT,24]

    in_maps = [
        {
            "atoms": atoms_np[k],
            "fragT": fragT_np[k],
            "meta": meta_np[k],
        }
        for k in range(NCORES)
    ]
    return in_maps, n_valid


_BUILT = None


def build_nc():
    global _BUILT
    if _BUILT is not None:
        return _BUILT
    nc = bacc.Bacc("TRN2", target_bir_lowering=False, debug=False)
    ins = {
        "atoms": nc.dram_tensor(
            "atoms", [ITERS, 128, CHUNKS * 128], BF16, kind="ExternalInput"
        ).ap(),
        "fragT": nc.dram_tensor(
            "fragT", [ITERS, 128, PAIRS * 128], BF16, kind="ExternalInput"
        ).ap(),
        "meta": nc.dram_tensor(
            "meta", [128, ITERS, 24], U8, kind="ExternalInput"
        ).ap(),
    }
    outs = {"out": nc.dram_tensor("out", [128, 1], F32, kind="ExternalOutput").ap()}
    with tile.TileContext(nc) as tc:
        build_body(tc, outs, ins)
    nc.compile()
    _fix_act_table_loads(nc)
    _BUILT = nc
    return nc


def _fix_act_table_loads(nc):
    """Collapse the Exp<->Ln table-load ping-pong into one load of
    natural_log_exp_and_others (serves Copy/Ln/Exp), saving ~1.3us per load."""
    from concourse.hw_specs import get_activation_tables

    tables = list(get_activation_tables(nc.m.arch).keys())
    target = tables.index("natural_log_exp_and_others")
    kept = False
    for f in nc.m.functions:
        for b in f.blocks:
            keep = []
            for i in b.instructions:
                if isinstance(i, mybir.InstLoadActFuncSet):
                    si = i.sync_info
                    assert si is None or (not si.on_wait and not si.on_update)
                    if kept:
                        continue
                    i.act_func_set_id = target
                    kept = True
                keep.append(i)
            b.instructions[:] = keep


def run_on_hw(in_maps, trace=False, **kw):
    nc = build_nc()
    return run_bass_kernel_spmd(nc, in_maps, list(range(NCORES)), trace=trace, **kw)


def kernel(**inputs) -> np.ndarray:
    in_maps, n_valid = prep_inputs(
        inputs["atom_embed"], inputs["fragment_embed"], inputs["atom2frag"]
    )
    res = run_on_hw(in_maps)
    total = 0.0
    for k in range(NCORES):
        total += float(np.asarray(res.results[k]["out"], dtype=np.float64).sum())
    if n_valid > 0:
        loss = np.float32(total / n_valid)
    else:
        loss = np.float32(0.0)
    return np.array(loss, dtype=np.float32)



# revision 18
# speedup vs baseline: 1.7016x; 1.7016x over previous
"""NodeContrastiveLoss Trainium2 kernel, v3.

Full inputs -> scalar loss, data-parallel over 8 NeuronCores (256 batches/core).

Math per batch (reference semantics):
  sums[f,d] = segment-sum of atom_embed over atom2frag   (onehot.T @ atoms)
  mn        = sums/||sums||  (count scale cancels; clamp via +eps on ssq)
  sims      = 10 * mn @ fn_hat.T
  per_frag  = ln(sum_g exp(sims)) - sims[f,f]
  loss      = sum(valid*per_frag)/n_valid

v3 design (vs v1 baseline, 183us):
  - All heavy inputs quantized host-side to fp8 e3m4 and shipped as ONE
    uint8 blob DMA per iteration (448KB): atoms + host-built one-hot +
    fragT (fn_hat.T * 10, folding the 1/temperature).
  - Segment matmuls run TRANSPOSED (lhsT=atoms chunk [a,128d] -> FWL,
    out d-major [128d, b, 64f]), killing v1's DVE one-hot build, the
    diag(D) scale trick, and the transpose matmuls.
  - ssq and pos(=diag sims_raw) computed as elementwise-square / mult
    (DVE, 4x mode) + ones-vector matmuls (free-size-1, 2-batch merged
    lhsT [128,128] -> FWL) instead of TTR/skewed-DMA diag extraction.
  - s = (ssq+eps)^-0.5 in one DVE tensor_scalar (add, pow).
  - exp(s*G) via 4 ACT Exp ops with per-partition scale AP, PSUM->PSUM,
    per-pair; Sum_g via one DVE tensor_reduce into an s_sel slab.
  - Tail (ln, *mask, reduce) deferred to one slab-wide pass at the end.

Per-iter engines (est): DMA 1.25us, PE ~1.9us, DVE ~1.6us, ACT ~1.2us.
Host: sum 8x128 partials (negated), divide by n_valid.
"""

import sys

sys.path.insert(0, "/opt/trn_rl_repo")

from contextlib import ExitStack

import ml_dtypes
import numpy as np

import concourse.bacc as bacc
import concourse.bass as bass
import concourse.tile as tile
from concourse import mybir
from concourse.bass_utils import run_bass_kernel_spmd

B, A, F_, D = 2048, 256, 64, 128
NCORES = 8
BPC = B // NCORES          # 256 batches per core
BPI = 8                    # batches per iteration
ITERS = BPC // BPI         # 32
PAIRS = BPI // 2           # 4
CHUNKS = BPI * 2           # 16 chunks of 128 atoms

BF16 = mybir.dt.bfloat16
F32 = mybir.dt.float32
U8 = mybir.dt.uint8
FP8 = mybir.dt.float8e3    # e3m4: 4 mantissa bits, range +-15.5
NP_FP8 = ml_dtypes.float8_e3m4
ALU = mybir.AluOpType
ACTF = mybir.ActivationFunctionType
AXIS = mybir.AxisListType

# blob layout per iter, bytes per partition
ATOMS_B = CHUNKS * 128     # 2048
OH_B = CHUNKS * 64         # 1024
FRAG_B = BPI * 64          # 512
BLOB_B = ATOMS_B + OH_B + FRAG_B  # 3584

EPS = 1e-12
# NOTE: tensor_tensor_reduce (custom DVE ucode) hangs the device under this
# runtime — stick to plain TT + tensor_reduce.
PROBE2_NO_ONES_MM = False
PROBE3_GUT_AFTER_COPY = False
PROBE4_NO_MATMUL = False


def build_body(tc, outs, ins):
    nc = tc.nc
    ctx = ExitStack()
    with ctx:
        const = ctx.enter_context(tc.tile_pool(name="const", bufs=1))
        dpool = ctx.enter_context(tc.tile_pool(name="dma", bufs=3))
        work = ctx.enter_context(tc.tile_pool(name="work", bufs=2))
        small = ctx.enter_context(tc.tile_pool(name="small", bufs=2))
        pseg = ctx.enter_context(tc.tile_pool(name="pseg", bufs=2, space="PSUM"))
        pg = ctx.enter_context(tc.tile_pool(name="pg", bufs=2, space="PSUM"))
        pe_ = ctx.enter_context(tc.tile_pool(name="pexp", bufs=2, space="PSUM"))
        psp = ctx.enter_context(tc.tile_pool(name="psp", bufs=2, space="PSUM"))

        # ---- one-time constants ----
        ones = const.tile([128, 1], BF16)
        nc.gpsimd.memset(ones[:], 1.0)
        eps_t = const.tile([128, 1], F32)
        nc.gpsimd.memset(eps_t[:], EPS)

        # slabs, fully written each run
        s_sel_slab = const.tile([128, ITERS, PAIRS], F32)
        s_slab = const.tile([128, ITERS, PAIRS], F32)
        pos_slab = const.tile([128, ITERS, PAIRS], F32)

        mask_slab = const.tile([128, ITERS, PAIRS], BF16)
        nc.sync.dma_start(mask_slab[:], ins["mask"])

        for it in range(ITERS):
            # ---- one blob DMA per iter ----
            blob = dpool.tile([128, BLOB_B], U8, tag="blob")
            nc.sync.dma_start(blob[:], ins["blob"][it])
            atoms = (
                blob[:, 0:ATOMS_B]
                .bitcast(FP8)
                .rearrange("p (c d) -> p c d", c=CHUNKS)
            )
            onehot = (
                blob[:, ATOMS_B : ATOMS_B + OH_B]
                .bitcast(FP8)
                .rearrange("p (c f) -> p c f", c=CHUNKS)
            )
            fragT = (
                blob[:, ATOMS_B + OH_B : BLOB_B]
                .bitcast(FP8)
                .rearrange("p (b g) -> p b g", b=BPI)
            )

            # ---- seg matmuls: psum_segT[:, b, :] = sum_h atoms_cT @ onehot_c
            # lhsT = atoms chunk [128a, 128d] (FWL), rhs = onehot [128a, 64f]
            sumsT = work.tile([128, BPI, 64], BF16, tag="sumsT")
            if PROBE4_NO_MATMUL:
                nc.vector.tensor_copy(
                    sumsT[:],
                    blob[:, 0:1024].bitcast(BF16).rearrange(
                        "p (b f) -> p b f", b=BPI
                    ),
                )
            else:
                psum_segT = pseg.tile([128, BPI, 64], F32, tag="segT")
                for b in range(BPI):
                    for h in range(2):
                        c = b * 2 + h
                        nc.tensor.matmul(
                            psum_segT[:, b, :],
                            atoms[:, c, :],
                            onehot[:, c, :],
                            start=(h == 0),
                            stop=(h == 1),
                        )
                # ---- sums -> SBUF bf16 (DVE copy; ACT is exp-bound) ----
                nc.vector.tensor_copy(sumsT[:], psum_segT[:])

            if PROBE3_GUT_AFTER_COPY:
                nc.vector.tensor_reduce(
                    s_slab[:, it, :],
                    sumsT[:].rearrange("p (j t) f -> p j (t f)", t=2),
                    axis=AXIS.X,
                    op=ALU.add,
                )
                nc.vector.tensor_reduce(
                    pos_slab[:, it, :],
                    fragT.rearrange("p (j t) f -> p j (t f)", t=2),
                    axis=AXIS.X,
                    op=ALU.add,
                )
                nc.vector.tensor_reduce(
                    s_sel_slab[:, it, :],
                    sumsT[:].rearrange("p (j t) f -> p j (t f)", t=2),
                    axis=AXIS.X,
                    op=ALU.max,
                )
                continue

            # ---- sq + h products (DVE 4x) ----
            sq = work.tile([128, BPI, 64], BF16, tag="sq")
            nc.vector.tensor_tensor(sq[:], sumsT[:], sumsT[:], op=ALU.mult)
            hprod = work.tile([128, BPI, 64], BF16, tag="h")
            nc.vector.tensor_tensor(hprod[:], sumsT[:], fragT, op=ALU.mult)

            # ---- ssq / pos via ones-matmuls (2-batch merged lhsT [128,128])
            # one PSUM tile (bank-granular alloc): [:,0,:]=ssq, [:,1,:]=pos
            psum_sp = psp.tile([128, 2, PAIRS], F32, tag="sp")
            if PROBE2_NO_ONES_MM:
                nc.vector.tensor_reduce(
                    psum_sp[:, 0, :],
                    sq[:].rearrange("p (j t) f -> p j (t f)", t=2),
                    axis=AXIS.X,
                    op=ALU.add,
                )
                nc.vector.tensor_reduce(
                    psum_sp[:, 1, :],
                    hprod[:].rearrange("p (j t) f -> p j (t f)", t=2),
                    axis=AXIS.X,
                    op=ALU.add,
                )
            else:
                for j in range(PAIRS):
                    nc.tensor.matmul(
                        psum_sp[:, 0, j : j + 1],
                        sq[:, 2 * j : 2 * j + 2, :].rearrange("p b f -> p (b f)"),
                        ones[:],
                        start=True,
                        stop=True,
                    )
                    nc.tensor.matmul(
                        psum_sp[:, 1, j : j + 1],
                        hprod[:, 2 * j : 2 * j + 2, :].rearrange("p b f -> p (b f)"),
                        ones[:],
                        start=True,
                        stop=True,
                    )

            # ---- s = exp(-0.5*ln(ssq+eps)) = rsqrt(ssq+eps) (ACT, one table)
            lnssq = small.tile([128, PAIRS], F32, tag="lnssq")
            nc.scalar.activation(lnssq[:], psum_sp[:, 0, :], ACTF.Ln, bias=eps_t[:])
            nc.scalar.activation(s_slab[:, it, :], lnssq[:], ACTF.Exp, scale=-0.5)
            nc.scalar.copy(pos_slab[:, it, :], psum_sp[:, 1, :])

            # ---- G matmuls: psum_G[:, j, :] rows 0:64 = G(tw=0), 64:128 = G(tw=1)
            psum_G = pg.tile([128, PAIRS, 64], F32, tag="G")
            for j in range(PAIRS):
                for tw in range(2):
                    nc.tensor.matmul(
                        psum_G[64 * tw : 64 * tw + 64, j, :],
                        sumsT[:, 2 * j + tw, :],
                        fragT[:, 2 * j + tw, :],
                        start=True,
                        stop=True,
                    )

            # ---- exp(s*G) per pair (ACT, per-partition scale AP) ----
            psum_exp = pe_.tile([128, PAIRS, 64], F32, tag="exp")
            for j in range(PAIRS):
                nc.scalar.activation(
                    psum_exp[:, j, :],
                    psum_G[:, j, :],
                    ACTF.Exp,
                    scale=s_slab[:, it, j : j + 1],
                )

            # ---- s_sel = sum_g exp -> slab (one DVE reduce) ----
            nc.vector.tensor_reduce(
                s_sel_slab[:, it, :], psum_exp[:], axis=AXIS.X, op=ALU.add
            )

        # ---- tail: -per_frag = s*pos - ln(s_sel); masked total ----
        ln_sel = const.tile([128, ITERS, PAIRS], F32)
        nc.scalar.activation(ln_sel[:], s_sel_slab[:], ACTF.Ln)
        # neg_pf = (pos * s) - ln_sel
        neg_pf = const.tile([128, ITERS, PAIRS], F32)
        spos = const.tile([128, ITERS, PAIRS], F32)
        nc.vector.tensor_tensor(spos[:], pos_slab[:], s_slab[:], op=ALU.mult)
        nc.vector.tensor_tensor(neg_pf[:], spos[:], ln_sel[:], op=ALU.subtract)
        junk = const.tile([128, ITERS, PAIRS], F32)
        outsb = const.tile([128, 1], F32)
        nc.vector.tensor_tensor(junk[:], neg_pf[:], mask_slab[:], op=ALU.mult)
        nc.vector.tensor_reduce(outsb[:], junk[:], axis=AXIS.XY, op=ALU.add)
        nc.sync.dma_start(outs["out"], outsb[:])


def prep_inputs(atom_embed, fragment_embed, atom2frag):
    """Host-side layout prep. Returns (in_maps, n_valid)."""
    am = np.asarray(atom_embed, dtype=np.float32)
    fe = np.asarray(fragment_embed, dtype=np.float32)
    af = np.asarray(atom2frag)

    # atoms fp8: [B,A,D] -> [core, it, p(=a%128), c(=b*2+h), d] bytes
    a5 = am.reshape(NCORES, ITERS, BPI, 2, 128, D)  # [nc, it, b, h, p, d]
    atoms8 = np.ascontiguousarray(
        a5.transpose(0, 1, 4, 2, 3, 5)
    ).astype(NP_FP8)  # [nc, it, p, b, h, d]
    atoms_bytes = atoms8.reshape(NCORES, ITERS, 128, ATOMS_B).view(np.uint8)

    # one-hot fp8: oh[nc, it, p, b, h, f] = (af_chunk[p] == f)
    i5 = af.reshape(NCORES, ITERS, BPI, 2, 128).transpose(0, 1, 4, 2, 3)
    # i5: [nc, it, p, b, h]
    oh = (i5[..., None] == np.arange(F_)[None, None, None, None, None, :]).astype(
        NP_FP8
    )
    oh_bytes = oh.reshape(NCORES, ITERS, 128, OH_B).view(np.uint8)

    # fragT fp8 (fn_hat.T * 10): [nc, it, p(=d), b, g]
    fen = 10.0 * fe / np.maximum(np.linalg.norm(fe, axis=-1, keepdims=True), 1e-8)
    f4 = fen.reshape(NCORES, ITERS, BPI, F_, D)
    fragT8 = np.ascontiguousarray(f4.transpose(0, 1, 4, 2, 3)).astype(NP_FP8)
    frag_bytes = fragT8.reshape(NCORES, ITERS, 128, FRAG_B).view(np.uint8)

    blob = np.concatenate([atoms_bytes, oh_bytes, frag_bytes], axis=-1)
    assert blob.shape == (NCORES, ITERS, 128, BLOB_B)

    # mask: [nc, p(=tw*64+f), it, j]
    counts = (af[:, :, None] == np.arange(F_)[None, None, :]).sum(axis=1)
    valid = counts > 0
    n_valid = int(valid.sum())
    v5 = valid.reshape(NCORES, ITERS, PAIRS, 2, F_)  # [nc, it, j, tw, f]
    mask_np = np.ascontiguousarray(
        v5.transpose(0, 3, 4, 1, 2).reshape(NCORES, 128, ITERS, PAIRS)
    ).astype(ml_dtypes.bfloat16)

    in_maps = [
        {"blob": blob[k], "mask": mask_np[k]}
        for k in range(NCORES)
    ]
    return in_maps, n_valid


_BUILT = None


def build_nc():
    global _BUILT
    if _BUILT is not None:
        return _BUILT
    nc = bacc.Bacc("TRN2", target_bir_lowering=False, debug=False)
    ins = {
        "blob": nc.dram_tensor(
            "blob", [ITERS, 128, BLOB_B], U8, kind="ExternalInput"
        ).ap(),
        "mask": nc.dram_tensor(
            "mask", [128, ITERS, PAIRS], BF16, kind="ExternalInput"
        ).ap(),
    }
    outs = {"out": nc.dram_tensor("out", [128, 1], F32, kind="ExternalOutput").ap()}
    with tile.TileContext(nc) as tc:
        build_body(tc, outs, ins)
    nc.compile()
    _fix_act_table_loads(nc)
    _BUILT = nc
    return nc


def _fix_act_table_loads(nc):
    """Collapse act-table loads into one load of natural_log_exp_and_others
    (serves Copy/Ln/Exp), saving ~1.3us per extra load."""
    from concourse.hw_specs import get_activation_tables

    tables = list(get_activation_tables(nc.m.arch).keys())
    target = tables.index("natural_log_exp_and_others")
    kept = False
    for f in nc.m.functions:
        for b in f.blocks:
            keep = []
            for i in b.instructions:
                if isinstance(i, mybir.InstLoadActFuncSet):
                    si = i.sync_info
                    assert si is None or (not si.on_wait and not si.on_update)
                    if kept:
                        continue
                    i.act_func_set_id = target
                    kept = True
                keep.append(i)
            b.instructions[:] = keep


def run_on_hw(in_maps, trace=False, **kw):
    nc = build_nc()
    return run_bass_kernel_spmd(nc, in_maps, list(range(NCORES)), trace=trace, **kw)


def kernel(**inputs) -> np.ndarray:
    in_maps, n_valid = prep_inputs(
        inputs["atom_embed"], inputs["fragment_embed"], inputs["atom2frag"]
    )
    res = run_on_hw(in_maps)
    total = 0.0
    for k in range(NCORES):
        # device accumulated (s*pos - ln(s_sel)) * mask = -sum(per_frag)
        total -= float(np.asarray(res.results[k]["out"], dtype=np.float64).sum())
    if n_valid > 0:
        loss = np.float32(total / n_valid)
    else:
        loss = np.float32(0.0)
    return np.array(loss, dtype=np.float32)
